# revision 1
# baseline (speedup 1.0000x reference)
"""DegreeAwareEdgeEncoder Trainium2 kernel (8 NeuronCores, Bass/Tile).

Sharding strategy (host side, inside kernel()):
  Edges are distributed core- and partition-parallel by *source-node range*
  (vertex-range / CSR-style partitioning): virtual node space of
  102400 = 8 cores x 128 partitions x 100 nodes; the edges whose src falls in
  partition slab (c, p)'s 100-node range are delivered to that slab, sorted by
  src.  A second copy of the dst column is distributed the same way by
  *dst*-range.  All arithmetic happens on the device:
    - out-degree per edge: per-partition local histogram of the slab's src
      values over its 100-node range (DVE dense compare; exact because all
      edges of one src node land in one slab) followed by an in-slab lookup.
    - in-degree: same histogram machinery on the dst-bucketed copy, AllGather
      of the 8 per-core [12800] slices into the full [102400] degree vector,
      int8 quad table, then a per-edge GPSIMD ap_gather + quad select.
    - output rows: du*A' + dv*B' + b with A'=W0+W2, B'=W1+W2 (PE computes the
      3xEMB coefficient rows; DVE does the broadcast expansion), written back
      as [E, 32] f32.
  The host only buckets/sorts (data layout), pads with sentinel edges, and
  inverts the layout permutation on the returned rows.
"""

import numpy as np

import concourse.bass as bass
import concourse.mybir as mybir
import concourse.tile as tile
from concourse.tile_rust import add_dep_helper
from concourse import bacc
from concourse.library_config import ap_gather as APG_LIB
from concourse.bass_utils import run_bass_kernel_spmd

# ---- constants ----
N_NODES = 100_000
N_EDGES = 3_200_000
EMB = 32
NCORES = 8
P = 128
BPP = 100                  # nodes per partition slab
NV = NCORES * P * BPP      # 102400 virtual nodes
RC = P * BPP               # 12800 nodes per core
T = 3584                   # slab capacity (cols per partition)
TQ = NV // 4               # 25600 int8 quads in the gather table
GCH = 16                   # ap_gather chunks
TCH = T // GCH             # 224 idx cols per chunk
NIC = TCH * 16             # 3584 idxs per chunk per q7 core
XCH = 56                   # expansion chunk cols
BCH = 4                    # hist bins per chunk
PAD_SENTINEL = BPP         # local value that never matches bins 0..99

f32 = mybir.dt.float32
i32 = mybir.dt.int32
i16 = mybir.dt.int16
i8 = mybir.dt.int8
AO = mybir.AluOpType

_CACHE = {}


def _build():
    nc = bacc.Bacc("TRN2", target_bir_lowering=False, debug=False,
                   num_devices=NCORES)

    psrc = nc.dram_tensor("psrc", [P, T], i32, kind="ExternalInput")
    pdst = nc.dram_tensor("pdst", [P, T], i32, kind="ExternalInput")
    sdst = nc.dram_tensor("sdst", [P, T], i32, kind="ExternalInput")
    wb_in = nc.dram_tensor("wb", [4, EMB], f32, kind="ExternalInput")
    mmat = nc.dram_tensor("mmat", [4, 4], f32, kind="ExternalInput")
    basec = nc.dram_tensor("basec", [P, 1], f32, kind="ExternalInput")
    iotab = nc.dram_tensor("iotab", [P, BPP], f32, kind="ExternalInput")
    smask = nc.dram_tensor("smask", [P, 16], f32, kind="ExternalInput")
    out = nc.dram_tensor("out", [P, T, EMB], f32, kind="ExternalOutput")

    slice_d = nc.dram_tensor("slice_d", [RC], f32)
    full_d = nc.dram_tensor("full_d", [NV], f32, addr_space="Shared")
    deg8_d = nc.dram_tensor("deg8_d", [NV], i8)
    abb_d = nc.dram_tensor("abb_d", [4, EMB], f32)

    with tile.TileContext(nc) as tc, nc.allow_low_precision(
            reason="all values are small integers, exact in bf16"):
        with (
            tc.tile_pool(name="main", bufs=1) as pool,
            tc.tile_pool(name="psum", bufs=1, space="PSUM") as psum,
        ):
            # ---- small constant inputs ----
            wb_t = pool.tile([4, EMB], f32)
            mm_t = pool.tile([4, 4], f32)
            basec_t = pool.tile([P, 1], f32)
            iotab_t = pool.tile([P, BPP], f32)
            nc.sync.dma_start(out=wb_t[:], in_=wb_in[:])
            nc.sync.dma_start(out=mm_t[:], in_=mmat[:])
            nc.sync.dma_start(out=basec_t[:], in_=basec[:])
            nc.sync.dma_start(out=iotab_t[:], in_=iotab[:])
            smask_t = pool.tile([P, 16], f32)
            nc.sync.dma_start(out=smask_t[:], in_=smask[:])

            # ---- coefficient rows: [A'; B'; b; 0] = mmat^T @ [W; b] ----
            abb_ps = psum.tile([4, EMB], f32)
            nc.tensor.matmul(out=abb_ps[:], lhsT=mm_t[:], rhs=wb_t[:],
                             start=True, stop=True)
            abb_t = pool.tile([4, EMB], f32)
            nc.vector.tensor_copy(out=abb_t[:], in_=abb_ps[:])
            nc.sync.dma_start(out=abb_d[:], in_=abb_t[:])
            arep = pool.tile([P, EMB], f32)
            brep = pool.tile([P, EMB], f32)
            crep = pool.tile([P, EMB], f32)
            nc.sync.dma_start(out=arep[:], in_=abb_d[0:1, :].to_broadcast([P, EMB]))
            nc.sync.dma_start(out=brep[:], in_=abb_d[1:2, :].to_broadcast([P, EMB]))
            nc.sync.dma_start(out=crep[:], in_=abb_d[2:3, :].to_broadcast([P, EMB]))

            hist_dst = pool.tile([P, BPP], f32)
            hist_src = pool.tile([P, BPP], f32)

            def dense_hist(vn, hist):
                for bc in range(BPP // BCH):
                    cmp = pool.tile([P, BCH, T], f32, tag="slotT")
                    nc.vector.tensor_tensor(
                        out=cmp[:],
                        in0=vn[:][:, None, :].to_broadcast([P, BCH, T]),
                        in1=iotab_t[:, BCH * bc:BCH * (bc + 1)][:, :, None]
                            .to_broadcast([P, BCH, T]),
                        op=AO.is_equal)
                    nc.vector.tensor_reduce(
                        out=hist[:, BCH * bc:BCH * (bc + 1)],
                        in_=cmp[:], op=AO.add, axis=mybir.AxisListType.X)

            # ---- dst histogram (slot B holds vndst) ----
            sdst_t = pool.tile([P, T], i32, tag="slotA")
            nc.sync.dma_start(out=sdst_t[:], in_=sdst[:])
            vndst = pool.tile([P, T], f32, tag="slotB")
            nc.vector.tensor_copy(out=vndst[:], in_=sdst_t[:])
            nc.vector.scalar_tensor_tensor(
                out=vndst[:], in0=vndst[:], scalar=basec_t[:, 0:1],
                in1=vndst[:], op0=AO.subtract, op1=AO.bypass)
            dense_hist(vndst, hist_dst)

            # ---- allgather in-degree slices ----
            nc.sync.dma_start(out=slice_d[:].rearrange("(p c) -> p c", p=P),
                              in_=hist_dst[:])
            nc.gpsimd.collective_compute(
                "AllGather", AO.bypass,
                replica_groups=[list(range(NCORES))],
                ins=[slice_d[:]], outs=[full_d[:]])

            # ---- src histogram + du lookup (slot B holds vnsrc) ----
            psrc_t = pool.tile([P, T], i32, tag="slotA")
            nc.sync.dma_start(out=psrc_t[:], in_=psrc[:])
            vnsrc = pool.tile([P, T], f32, tag="slotB")
            nc.vector.tensor_copy(out=vnsrc[:], in_=psrc_t[:])
            nc.vector.scalar_tensor_tensor(
                out=vnsrc[:], in0=vnsrc[:], scalar=basec_t[:, 0:1],
                in1=vnsrc[:], op0=AO.subtract, op1=AO.bypass)
            dense_hist(vnsrc, hist_src)
            du_t = pool.tile([P, T], mybir.dt.bfloat16)
            nc.vector.memset(du_t[:], 0.0)
            for bc in range(BPP // BCH):
                cmp = pool.tile([P, BCH, T], f32, tag="slotT")
                nc.vector.tensor_tensor(
                    out=cmp[:],
                    in0=vnsrc[:][:, None, :].to_broadcast([P, BCH, T]),
                    in1=iotab_t[:, BCH * bc:BCH * (bc + 1)][:, :, None]
                        .to_broadcast([P, BCH, T]),
                    op=AO.is_equal)
                for j in range(BCH):
                    b = BCH * bc + j
                    nc.vector.scalar_tensor_tensor(
                        out=du_t[:], in0=cmp[:, j, :],
                        scalar=hist_src[:, b:b + 1], in1=du_t[:],
                        op0=AO.mult, op1=AO.add)

            # ---- int8 degree table, replicated per partition ----
            degf = pool.tile([P, NV // P], f32, tag="slotE")
            nc.sync.dma_start(out=degf[:],
                              in_=full_d[:].rearrange("(p c) -> p c", p=P))
            deg8s = pool.tile([P, NV // P], i8, tag="wsel")
            nc.vector.tensor_copy(out=deg8s[:], in_=degf[:])
            nc.sync.dma_start(out=deg8_d[:].rearrange("(p c) -> p c", p=P),
                              in_=deg8s[:])
            table8 = pool.tile([P, NV], i8, tag="slotT")
            nc.sync.dma_start(
                out=table8[:],
                in_=deg8_d[:][None, :].to_broadcast([P, NV]))

            # ---- gather indices: quad idx int16 + remainder ----
            pdst_t = pool.tile([P, T], i32, tag="slotA")
            nc.sync.dma_start(out=pdst_t[:], in_=pdst[:])
            pf = pool.tile([P, T], f32, tag="slotB")
            nc.vector.tensor_copy(out=pf[:], in_=pdst_t[:])
            qf = pool.tile([P, T], f32, tag="slotE")
            nc.vector.tensor_scalar(out=qf[:], in0=pf[:], scalar1=0.25,
                                    scalar2=-0.375, op0=AO.mult, op1=AO.add)
            idxw = pool.tile([P, T], i16)
            nc.vector.tensor_copy(out=idxw[:], in_=qf[:])   # round -> exact quad
            qround = pool.tile([P, T], f32, tag="slotE")
            nc.vector.tensor_copy(out=qround[:], in_=idxw[:])
            rem = pf                                        # dst - 4*quad in 0..3
            nc.vector.scalar_tensor_tensor(
                out=rem[:], in0=qround[:], scalar=-4.0, in1=pf[:],
                op0=AO.mult, op1=AO.add)

            # ---- per-edge in-degree gather (GPSIMD ap_gather, int8 quads) ----
            lib_inst = nc.gpsimd.load_library(APG_LIB)
            tbl_q = table8[:].rearrange("p (q d) -> p q d", d=4)
            dv_t = pool.tile([P, T], mybir.dt.bfloat16)
            iota4 = pool.tile([P, 4], f32)
            for r in range(4):
                nc.vector.memset(iota4[:, r:r + 1], float(r))
            bf = mybir.dt.bfloat16
            for g in range(GCH):
                gsl = slice(g * TCH, (g + 1) * TCH)
                qgat = pool.tile([P, NIC, 4], i8, tag="slotA")
                gat_inst = nc.gpsimd.ap_gather(
                    qgat[:], tbl_q, idxw[:, g * TCH:(g + 1) * TCH],
                    P, TQ, 4, NIC)
                add_dep_helper(gat_inst.ins, lib_inst.ins, sync=True,
                               reason="ap_gather needs library loaded")
                # out[p, 16t+c, r] holds, for every partition p of group k, the
                # quad bytes of edge (16k+c, t).  Partition p wants c == p%16:
                # dense mask-select on full partitions.
                qbf = pool.tile([P, NIC, 4], bf, tag="qbf")
                nc.vector.tensor_copy(out=qbf[:], in_=qgat[:])
                qv = qbf[:].rearrange("p (t c) r -> p t c r", c=16)
                nc.vector.tensor_tensor(
                    out=qv,
                    in0=qv,
                    in1=smask_t[:][:, None, :, None].to_broadcast([P, TCH, 16, 4]),
                    op=AO.mult)
                # reduce over c (strided innermost view): [p, t, r, c]
                wsel = pool.tile([P, TCH, 4], bf, tag="wsel")
                qcv = qbf[:].rearrange("p (t c) r -> p t r c", c=16)
                nc.vector.tensor_reduce(out=wsel[:], in_=qcv,
                                        op=AO.add, axis=mybir.AxisListType.X)
                # select quad byte r = rem
                maskr = pool.tile([P, TCH, 4], bf, tag="maskr")
                nc.vector.tensor_tensor(
                    out=maskr[:],
                    in0=rem[:, gsl][:, :, None].to_broadcast([P, TCH, 4]),
                    in1=iota4[:][:, None, :].to_broadcast([P, TCH, 4]),
                    op=AO.is_equal)
                nc.vector.tensor_tensor(out=maskr[:], in0=maskr[:],
                                        in1=wsel[:], op=AO.mult)
                nc.vector.tensor_reduce(out=dv_t[:, gsl], in_=maskr[:],
                                        op=AO.add, axis=mybir.AxisListType.X)

            # ---- expansion: out = du*A' + dv*B' + b ----
            for x in range(T // XCH):
                sl = slice(x * XCH, (x + 1) * XCH)
                xt = pool.tile([P, XCH, EMB], f32, tag="slotE")
                xo = pool.tile([P, XCH, EMB], f32, tag="slotX")
                duf = pool.tile([P, XCH], f32, tag="duf")
                dvf = pool.tile([P, XCH], f32, tag="dvf")
                nc.vector.tensor_copy(out=duf[:], in_=du_t[:, sl])
                nc.vector.tensor_copy(out=dvf[:], in_=dv_t[:, sl])
                nc.vector.tensor_tensor(
                    out=xt[:],
                    in0=duf[:][:, :, None].to_broadcast([P, XCH, EMB]),
                    in1=arep[:][:, None, :].to_broadcast([P, XCH, EMB]),
                    op=AO.mult)
                nc.vector.tensor_tensor(
                    out=xo[:],
                    in0=dvf[:][:, :, None].to_broadcast([P, XCH, EMB]),
                    in1=brep[:][:, None, :].to_broadcast([P, XCH, EMB]),
                    op=AO.mult)
                nc.vector.tensor_tensor(out=xo[:], in0=xo[:], in1=xt[:],
                                        op=AO.add)
                nc.vector.tensor_tensor(
                    out=xo[:], in0=xo[:],
                    in1=crep[:][:, None, :].to_broadcast([P, XCH, EMB]),
                    op=AO.add)
                nc.scalar.dma_start(out=out[:, sl, :], in_=xo[:])

    nc.compile()
    return nc


def _host_prep(edge_index, W, b):
    src = np.asarray(edge_index[0], dtype=np.int64).astype(np.int32)
    dst = np.asarray(edge_index[1], dtype=np.int64).astype(np.int32)
    E = src.shape[0]

    def bucketize(keys, other):
        """Distribute edges to (core, partition, col) slabs by key//BPP."""
        order = np.argsort(keys, kind="stable")
        k_s = keys[order]
        o_s = other[order] if other is not None else None
        part = (k_s // BPP).astype(np.int64)          # 0..1023 global partition
        counts = np.bincount(part, minlength=NCORES * P)
        if counts.max() > T:
            raise RuntimeError(f"slab overflow: {counts.max()} > {T}")
        starts = np.zeros(NCORES * P + 1, np.int64)
        np.cumsum(counts, out=starts[1:])
        # position of each edge within its slab
        pos_in_slab = np.arange(E, dtype=np.int64) - starts[part]
        key_arr = np.full((NCORES * P, T), -1, np.int32)
        key_arr[part, pos_in_slab] = k_s
        oth_arr = None
        if o_s is not None:
            oth_arr = np.full((NCORES * P, T), N_NODES, np.int32)
            oth_arr[part, pos_in_slab] = o_s
        # sentinel for key: base + BPP (never matches local bins 0..99)
        gp = np.arange(NCORES * P, dtype=np.int32)
        pad_val = (gp * BPP + BPP)[:, None].astype(np.int32)
        key_arr = np.where(key_arr < 0, pad_val, key_arr)
        return key_arr.reshape(NCORES, P, T), \
            (oth_arr.reshape(NCORES, P, T) if oth_arr is not None else None), \
            order, counts.reshape(NCORES, P)

    psrc_a, pdst_a, order1, counts1 = bucketize(src, dst)
    sdst_a, _, _, _ = bucketize(dst, None)

    wb = np.concatenate([np.asarray(W, np.float32),
                         np.asarray(b, np.float32)[None, :]], axis=0)
    # [A'; B'; b; 0] = mmat^T @ [W0; W1; W2; b]
    mmat = np.array([[1, 0, 0, 0],
                     [0, 1, 0, 0],
                     [1, 1, 0, 0],
                     [0, 0, 1, 0]], np.float32)
    iota_row = np.tile(np.arange(BPP, dtype=np.float32), (P, 1))
    smask_a = (np.arange(16)[None, :] == (np.arange(P) % 16)[:, None]
               ).astype(np.float32)
    in_maps = []
    for c in range(NCORES):
        basec_c = ((c * P + np.arange(P)) * BPP).astype(np.float32)[:, None]
        in_maps.append({
            "psrc": psrc_a[c], "pdst": pdst_a[c], "sdst": sdst_a[c],
            "wb": wb, "mmat": mmat, "basec": basec_c, "iotab": iota_row,
            "smask": smask_a,
        })
    return in_maps, order1, counts1


def kernel(edge_index, num_nodes, W, b):
    global _CACHE
    if "nc" not in _CACHE:
        _CACHE["nc"] = _build()
    nc = _CACHE["nc"]

    in_maps, order1, counts1 = _host_prep(edge_index, W, b)
    res = run_bass_kernel_spmd(nc, in_maps, list(range(NCORES)))

    E = np.asarray(edge_index[0]).shape[0]
    out_full = np.empty((E, EMB), np.float32)
    # rows in (core, partition, col) order, real rows only, equal order1 order
    rows = []
    for c in range(NCORES):
        o = res.results[c]["out"]          # [P, T, EMB]
        for p in range(P):
            n = counts1[c, p]
            if n:
                rows.append(o[p, :n, :])
    out_full[order1] = np.concatenate(rows, axis=0)
    return out_full



# revision 8
# speedup vs baseline: 2.4389x; 2.4389x over previous
"""DegreeAwareEdgeEncoder Trainium2 kernel (8 NeuronCores, Bass/Tile).

Sharding strategy (host side, inside kernel()):
  Edges are distributed core- and partition-parallel by *source-node range*
  (vertex-range partitioning): virtual node space of 102400 = 8 cores x 128
  partitions x 100 nodes; the edges whose src falls in partition slab
  (c, p)'s 100-node range are delivered to that slab, sorted by src.  A
  second copy of the dst column is distributed the same way by *dst*-range.
  All arithmetic happens on the device:
    - out-degree per edge (du): the slab's src column is sorted, so each
      node's edges form a run; run start/end positions are computed with a
      log2 shifted max/min window on DVE and du = end - start + 1.
    - in-degree per node: on the dst-sorted copy, run-boundary first/last
      positions are scattered per local node via GPSIMD local_scatter; the
      difference is the local in-degree histogram (device segment count).
      The 8 per-core [12800] int8 slices are AllGathered into the full
      [102400] degree vector.
    - in-degree per edge (dv): int8 quad table gather (GPSIMD ap_gather)
      followed by a bit-exact diagonal extract (bitwise_and mask +
      bitwise_or reduce + per-edge shift) on DVE.
    - output rows: du*A' + dv*B' + b with A'=W0+W2, B'=W1+W2 (PE computes
      the coefficient rows), expanded in fp16 [P, EMB, chunk] tiles and
      written back chunk-major.
  The host only buckets/sorts (data layout), pads with sentinel edges, and
  inverts the layout permutation on the returned rows.
"""

import numpy as np

import concourse.bass as bass
import concourse.mybir as mybir
import concourse.tile as tile
from concourse.tile_rust import add_dep_helper
from concourse import bacc
from concourse.library_config import ap_gather as APG_LIB
from concourse.library_config import local_scatter as LS_LIB
from concourse.bass_utils import run_bass_kernel_spmd

# ---- constants ----
N_NODES = 100_000
N_EDGES = 3_200_000
EMB = 32
NCORES = 8
P = 128
BPP = 100                  # nodes per partition slab
NV = NCORES * P * BPP      # 102400 virtual nodes
RC = P * BPP               # 12800 nodes per core
T = 3584                   # slab capacity (cols per partition)
TQ = NV // 4               # 25600 int8 quads in the gather table
GCH = 16                   # ap_gather chunks
TCH = T // GCH             # 224 idx cols per chunk
NIC = TCH * 16             # 3584 idxs per chunk per q7 core
XCH = 64                   # expansion chunk cols
NX = T // XCH              # 56 expansion chunks
BIG = 65536.0              # larger than any position

f32 = mybir.dt.float32
f16 = mybir.dt.float16
i32 = mybir.dt.int32
i16 = mybir.dt.int16
i8 = mybir.dt.int8
AO = mybir.AluOpType
AX = mybir.AxisListType

_CACHE = {}


def _build():
    nc = bacc.Bacc("TRN2", target_bir_lowering=False, debug=False,
                   num_devices=NCORES)

    psrc = nc.dram_tensor("psrc", [P, T], i32, kind="ExternalInput")
    pdst = nc.dram_tensor("pdst", [P, T], i32, kind="ExternalInput")
    sdst = nc.dram_tensor("sdst", [P, T], i32, kind="ExternalInput")
    wb_in = nc.dram_tensor("wb", [4, EMB], f32, kind="ExternalInput")
    mmat = nc.dram_tensor("mmat", [4, 4], f32, kind="ExternalInput")
    basec = nc.dram_tensor("basec", [P, 1], f32, kind="ExternalInput")
    iota16 = nc.dram_tensor("iota16", [P, T], i16, kind="ExternalInput")
    smask = nc.dram_tensor("smask", [P, 16], i32, kind="ExternalInput")
    out = nc.dram_tensor("out", [P, NX, EMB, XCH], f16, kind="ExternalOutput")

    slice_d = nc.dram_tensor("slice_d", [RC], i8)
    full_d = nc.dram_tensor("full_d", [NV], i8, addr_space="Shared")
    abb_d = nc.dram_tensor("abb_d", [4, EMB], f16)

    with tile.TileContext(nc) as tc, nc.allow_low_precision(
            reason="degrees are small integers, exact in fp16; fp16 products "
                   "are within the 2e-2 relative-error gate"):
        with (
            tc.tile_pool(name="persist", bufs=1) as pp,
            tc.tile_pool(name="psum", bufs=1, space="PSUM") as psum,
        ):
            # ---- persistent tiles ----
            wb_t = pp.tile([4, EMB], f32)
            mm_t = pp.tile([4, 4], f32)
            basec_t = pp.tile([P, 1], f32)
            smask_t = pp.tile([P, 16], i32)
            nc.sync.dma_start(out=wb_t[:], in_=wb_in[:])
            nc.sync.dma_start(out=mm_t[:], in_=mmat[:])
            nc.sync.dma_start(out=basec_t[:], in_=basec[:])
            nc.sync.dma_start(out=smask_t[:], in_=smask[:])

            # coefficient rows: [A'; B'; b; 0] = mmat^T @ [W; b]
            abb_ps = psum.tile([4, EMB], f32)
            nc.tensor.matmul(out=abb_ps[:], lhsT=mm_t[:], rhs=wb_t[:],
                             start=True, stop=True)
            abb_h = pp.tile([4, EMB], f16)
            nc.vector.tensor_copy(out=abb_h[:], in_=abb_ps[:])
            nc.sync.dma_start(out=abb_d[:], in_=abb_h[:])
            arow = pp.tile([P, EMB], f16)
            brow = pp.tile([P, EMB], f16)
            crow = pp.tile([P, EMB], f16)
            nc.sync.dma_start(out=arow[:], in_=abb_d[0:1, :].to_broadcast([P, EMB]))
            nc.sync.dma_start(out=brow[:], in_=abb_d[1:2, :].to_broadcast([P, EMB]))
            nc.sync.dma_start(out=crow[:], in_=abb_d[2:3, :].to_broadcast([P, EMB]))
            arep = pp.tile([P, EMB, XCH], f16)
            brep = pp.tile([P, EMB, XCH], f16)
            crep = pp.tile([P, EMB, XCH], f16)
            nc.vector.tensor_copy(
                out=arep[:], in_=arow[:][:, :, None].to_broadcast([P, EMB, XCH]))
            nc.vector.tensor_copy(
                out=brep[:], in_=brow[:][:, :, None].to_broadcast([P, EMB, XCH]))
            nc.vector.tensor_copy(
                out=crep[:], in_=crow[:][:, :, None].to_broadcast([P, EMB, XCH]))

            idxw = pp.tile([P, T], i16)
            rem8 = pp.tile([P, T], i16)
            du_h = pp.tile([P, T], f16)
            dv_h = pp.tile([P, T], f16)

            ls_lib = nc.gpsimd.load_library(LS_LIB)

            with tc.tile_pool(name="early", bufs=1) as pe:
                iota16_t = pe.tile([P, T], i16, tag="io0")
                nc.sync.dma_start(out=iota16_t[:], in_=iota16[:])

                # ==== dst phase: local in-degree histogram slice ====
                sdst_t = pe.tile([P, T], i32, tag="A")
                nc.sync.dma_start(out=sdst_t[:], in_=sdst[:])
                vds = pe.tile([P, T], f32, tag="B")
                nc.vector.tensor_copy(out=vds[:], in_=sdst_t[:])
                nc.vector.scalar_tensor_tensor(
                    out=vds[:], in0=vds[:], scalar=basec_t[:, 0:1],
                    in1=vds[:], op0=AO.subtract, op1=AO.bypass)
                # run starts
                brkF = pe.tile([P, T], f32, tag="C")
                nc.vector.memset(brkF[:, 0:1], 1.0)
                nc.vector.tensor_tensor(out=brkF[:, 1:], in0=vds[:, 1:],
                                        in1=vds[:, :T - 1], op=AO.not_equal)
                # idxF = vds*brkF + brkF - 1  (v at run starts, -1 elsewhere)
                tmpF = pe.tile([P, T], f32, tag="D")
                nc.vector.tensor_tensor(out=tmpF[:], in0=vds[:], in1=brkF[:],
                                        op=AO.mult)
                nc.vector.tensor_tensor(out=tmpF[:], in0=tmpF[:], in1=brkF[:],
                                        op=AO.add)
                nc.vector.tensor_scalar(out=tmpF[:], in0=tmpF[:], scalar1=-1.0,
                                        scalar2=None, op0=AO.add)
                idxF = pe.tile([P, T], i16, tag="E")
                nc.vector.tensor_copy(out=idxF[:], in_=tmpF[:])
                Fst = pe.tile([P, BPP], i16)
                s1 = nc.gpsimd.local_scatter(Fst[:], iota16_t[:], idxF[:],
                                             P, BPP, T)
                add_dep_helper(s1.ins, ls_lib.ins, sync=True,
                               reason="local_scatter needs library loaded")
                # run ends
                brkL = pe.tile([P, T], f32, tag="F")
                nc.vector.memset(brkL[:, T - 1:T], 1.0)
                nc.vector.tensor_tensor(out=brkL[:, :T - 1], in0=vds[:, :T - 1],
                                        in1=vds[:, 1:], op=AO.not_equal)
                tmpL = pe.tile([P, T], f32, tag="G")
                nc.vector.tensor_tensor(out=tmpL[:], in0=vds[:], in1=brkL[:],
                                        op=AO.mult)
                nc.vector.tensor_tensor(out=tmpL[:], in0=tmpL[:], in1=brkL[:],
                                        op=AO.add)
                nc.vector.tensor_scalar(out=tmpL[:], in0=tmpL[:], scalar1=-1.0,
                                        scalar2=None, op0=AO.add)
                idxL = pe.tile([P, T], i16, tag="H")
                nc.vector.tensor_copy(out=idxL[:], in_=tmpL[:])
                # Ldata = t + 1 (reuses idxF's buffer once s1 has read it)
                Ldata = pe.tile([P, T], i16, tag="E")
                nc.vector.tensor_scalar(out=Ldata[:], in0=iota16_t[:],
                                        scalar1=1, scalar2=None, op0=AO.add)
                Lst = pe.tile([P, BPP], i16)
                s2 = nc.gpsimd.local_scatter(Lst[:], Ldata[:], idxL[:],
                                             P, BPP, T)
                add_dep_helper(s2.ins, ls_lib.ins, sync=True,
                               reason="local_scatter needs library loaded")
                degL = pe.tile([P, BPP], f32)
                degF = pe.tile([P, BPP], f32)
                nc.vector.tensor_copy(out=degL[:], in_=Lst[:])
                nc.vector.tensor_copy(out=degF[:], in_=Fst[:])
                nc.vector.tensor_tensor(out=degL[:], in0=degL[:], in1=degF[:],
                                        op=AO.subtract)
                deg8 = pe.tile([P, BPP], i8)
                nc.vector.tensor_copy(out=deg8[:], in_=degL[:])
                nc.sync.dma_start(
                    out=slice_d[:].rearrange("(p c) -> p c", p=P),
                    in_=deg8[:])
                nc.gpsimd.collective_compute(
                    "AllGather", AO.bypass,
                    replica_groups=[list(range(NCORES))],
                    ins=[slice_d[:]], outs=[full_d[:]])

                # ==== src phase: du per edge via run windows ====
                psrc_t = pe.tile([P, T], i32, tag="A")
                nc.sync.dma_start(out=psrc_t[:], in_=psrc[:])
                vsf = pe.tile([P, T], f32, tag="B")
                nc.vector.tensor_copy(out=vsf[:], in_=psrc_t[:])
                nc.vector.scalar_tensor_tensor(
                    out=vsf[:], in0=vsf[:], scalar=basec_t[:, 0:1],
                    in1=vsf[:], op0=AO.subtract, op1=AO.bypass)
                iotaF = pe.tile([P, T], f32, tag="C")
                nc.vector.tensor_copy(out=iotaF[:], in_=iota16_t[:])
                # S side: most recent run start within window 128
                brkS = pe.tile([P, T], f32, tag="D")
                nc.vector.memset(brkS[:, 0:1], 1.0)
                nc.vector.tensor_tensor(out=brkS[:, 1:], in0=vsf[:, 1:],
                                        in1=vsf[:, :T - 1], op=AO.not_equal)
                sa = pe.tile([P, T], f32, tag="E2")
                sb = pe.tile([P, T], f32, tag="F")
                # S0 = iotaF*brk + BIG*brk - BIG
                nc.vector.tensor_scalar(out=sa[:], in0=brkS[:], scalar1=BIG,
                                        scalar2=-BIG, op0=AO.mult, op1=AO.add)
                nc.vector.tensor_tensor(out=sb[:], in0=iotaF[:], in1=brkS[:],
                                        op=AO.mult)
                nc.vector.tensor_tensor(out=sa[:], in0=sa[:], in1=sb[:],
                                        op=AO.add)
                cur, nxt = sa, sb
                for s in (1, 2, 4, 8, 16, 32, 64):
                    nc.vector.tensor_tensor(out=nxt[:, s:], in0=cur[:, s:],
                                            in1=cur[:, :T - s], op=AO.max)
                    nc.vector.tensor_copy(out=nxt[:, :s], in_=cur[:, :s])
                    cur, nxt = nxt, cur
                S_fin = cur
                # E side: earliest run end within window 128
                brkE = pe.tile([P, T], f32, tag="D2")
                nc.vector.memset(brkE[:, T - 1:T], 1.0)
                nc.vector.tensor_tensor(out=brkE[:, :T - 1], in0=vsf[:, :T - 1],
                                        in1=vsf[:, 1:], op=AO.not_equal)
                ea = pe.tile([P, T], f32, tag="G")
                eb = pe.tile([P, T], f32, tag="H2")
                # E0 = iotaF*brkE - BIG*brkE + BIG
                nc.vector.tensor_scalar(out=ea[:], in0=brkE[:], scalar1=-BIG,
                                        scalar2=BIG, op0=AO.mult, op1=AO.add)
                nc.vector.tensor_tensor(out=eb[:], in0=iotaF[:], in1=brkE[:],
                                        op=AO.mult)
                nc.vector.tensor_tensor(out=ea[:], in0=ea[:], in1=eb[:],
                                        op=AO.add)
                cur, nxt = ea, eb
                for s in (1, 2, 4, 8, 16, 32, 64):
                    nc.vector.tensor_tensor(out=nxt[:, :T - s],
                                            in0=cur[:, :T - s],
                                            in1=cur[:, s:], op=AO.min)
                    nc.vector.tensor_copy(out=nxt[:, T - s:], in_=cur[:, T - s:])
                    cur, nxt = nxt, cur
                E_fin = cur
                duf = pe.tile([P, T], f32, tag="C")
                nc.vector.tensor_tensor(out=duf[:], in0=E_fin[:], in1=S_fin[:],
                                        op=AO.subtract)
                nc.vector.tensor_scalar(out=du_h[:], in0=duf[:], scalar1=1.0,
                                        scalar2=None, op0=AO.add)

                # ==== dv idx prep ====
                pdst_t = pe.tile([P, T], i32, tag="A")
                nc.sync.dma_start(out=pdst_t[:], in_=pdst[:])
                pf = pe.tile([P, T], f32, tag="B")
                nc.vector.tensor_copy(out=pf[:], in_=pdst_t[:])
                qf = pe.tile([P, T], f32, tag="D")
                nc.vector.tensor_scalar(out=qf[:], in0=pf[:], scalar1=0.25,
                                        scalar2=-0.375, op0=AO.mult, op1=AO.add)
                nc.vector.tensor_copy(out=idxw[:], in_=qf[:])  # round -> quad
                qround = pe.tile([P, T], f32, tag="D2")
                nc.vector.tensor_copy(out=qround[:], in_=idxw[:])
                remf = pe.tile([P, T], f32, tag="E2")
                # remf = (pf - 4*qround) * 8 = per-edge shift amounts
                nc.vector.scalar_tensor_tensor(
                    out=remf[:], in0=qround[:], scalar=-4.0, in1=pf[:],
                    op0=AO.mult, op1=AO.add)
                nc.vector.tensor_scalar(out=remf[:], in0=remf[:], scalar1=8.0,
                                        scalar2=None, op0=AO.mult)
                nc.vector.tensor_copy(out=rem8[:], in_=remf[:])

            # ==== late phase: table, gather, extract, expansion ====
            with tc.tile_pool(name="late", bufs=1) as pl:
                table8 = pl.tile([P, NV], i8, tag="table")
                nc.sync.dma_start(
                    out=table8[:],
                    in_=full_d[:][None, :].to_broadcast([P, NV]))
                ag_lib = nc.gpsimd.load_library(APG_LIB)
                add_dep_helper(ag_lib.ins, s1.ins, sync=True,
                               reason="library swap after scatters done")
                add_dep_helper(ag_lib.ins, s2.ins, sync=True,
                               reason="library swap after scatters done")
                tbl_q = table8[:].rearrange("p (q d) -> p q d", d=4)
                for g in range(GCH):
                    gsl = slice(g * TCH, (g + 1) * TCH)
                    qgat = pl.tile([P, NIC, 4], i8, tag=f"qg{g % 2}")
                    gat = nc.gpsimd.ap_gather(
                        qgat[:], tbl_q, idxw[:, gsl], P, TQ, 4, NIC)
                    add_dep_helper(gat.ins, ag_lib.ins, sync=True,
                                   reason="ap_gather needs library loaded")
                    qw = qgat[:].rearrange("p n d -> p (n d)").bitcast(i32)
                    qv = qw.rearrange("p (t c) -> p t c", c=16)
                    # bit-exact diagonal extract: AND-mask, OR-reduce
                    nc.vector.tensor_tensor(
                        out=qv, in0=qv,
                        in1=smask_t[:][:, None, :].to_broadcast([P, TCH, 16]),
                        op=AO.bitwise_and)
                    wsel = pl.tile([P, TCH], i32, tag=f"ws{g % 2}")
                    nc.vector.tensor_reduce(out=wsel[:], in_=qv,
                                            op=AO.bitwise_or, axis=AX.X)
                    shc = pl.tile([P, TCH], i32, tag=f"sh{g % 2}")
                    nc.vector.tensor_copy(out=shc[:], in_=rem8[:, gsl])
                    nc.vector.tensor_tensor(out=wsel[:], in0=wsel[:],
                                            in1=shc[:],
                                            op=AO.logical_shift_right)
                    nc.vector.tensor_scalar(out=wsel[:], in0=wsel[:],
                                            scalar1=255, scalar2=None,
                                            op0=AO.bitwise_and)
                    nc.vector.tensor_copy(out=dv_h[:, gsl], in_=wsel[:])

                # ==== expansion: out = du*A' + dv*B' + b ====
                for x in range(NX):
                    sl = slice(x * XCH, (x + 1) * XCH)
                    x1 = pl.tile([P, EMB, XCH], f16, tag=f"ex{x % 2}")
                    x2 = pl.tile([P, EMB, XCH], f16, tag="ey")
                    nc.vector.tensor_tensor(
                        out=x1[:],
                        in0=du_h[:, sl][:, None, :].to_broadcast([P, EMB, XCH]),
                        in1=arep[:], op=AO.mult)
                    nc.vector.tensor_tensor(
                        out=x2[:],
                        in0=dv_h[:, sl][:, None, :].to_broadcast([P, EMB, XCH]),
                        in1=brep[:], op=AO.mult)
                    nc.vector.tensor_tensor(out=x1[:], in0=x1[:], in1=x2[:],
                                            op=AO.add)
                    nc.vector.tensor_tensor(out=x1[:], in0=x1[:], in1=crep[:],
                                            op=AO.add)
                    nc.scalar.dma_start(out=out[:, x, :, :], in_=x1[:])

    nc.compile()
    return nc


def _host_prep(edge_index, W, b):
    src = np.asarray(edge_index[0], dtype=np.int64).astype(np.int32)
    dst = np.asarray(edge_index[1], dtype=np.int64).astype(np.int32)
    E = src.shape[0]

    def bucketize(keys, other, pad_delta):
        """Distribute edges to (core, partition, col) slabs by key//BPP."""
        order = np.argsort(keys, kind="stable")
        k_s = keys[order]
        o_s = other[order] if other is not None else None
        part = (k_s // BPP).astype(np.int64)          # 0..1023 global partition
        counts = np.bincount(part, minlength=NCORES * P)
        if counts.max() > T:
            raise RuntimeError(f"slab overflow: {counts.max()} > {T}")
        starts = np.zeros(NCORES * P + 1, np.int64)
        np.cumsum(counts, out=starts[1:])
        pos_in_slab = np.arange(E, dtype=np.int64) - starts[part]
        key_arr = np.full((NCORES * P, T), -(1 << 30), np.int32)
        key_arr[part, pos_in_slab] = k_s
        oth_arr = None
        if o_s is not None:
            oth_arr = np.zeros((NCORES * P, T), np.int32)
            oth_arr[part, pos_in_slab] = o_s
        # pad sentinel: base + pad_delta (outside the 0..99 local range)
        gp = np.arange(NCORES * P, dtype=np.int32)
        pad_val = (gp * BPP + pad_delta)[:, None].astype(np.int32)
        key_arr = np.where(key_arr < -(1 << 29), pad_val, key_arr)
        return key_arr.reshape(NCORES, P, T), \
            (oth_arr.reshape(NCORES, P, T) if oth_arr is not None else None), \
            order, counts.reshape(NCORES, P)

    psrc_a, pdst_a, order1, counts1 = bucketize(src, dst, BPP)
    sdst_a, _, _, _ = bucketize(dst, None, -1)

    wb = np.concatenate([np.asarray(W, np.float32),
                         np.asarray(b, np.float32)[None, :]], axis=0)
    # [A'; B'; b; 0] = mmat^T @ [W0; W1; W2; b]
    mmat = np.array([[1, 0, 0, 0],
                     [0, 1, 0, 0],
                     [1, 1, 0, 0],
                     [0, 0, 1, 0]], np.float32)
    iota_a = np.tile(np.arange(T, dtype=np.int16), (P, 1))
    smask_a = np.where(np.arange(16)[None, :] == (np.arange(P) % 16)[:, None],
                       -1, 0).astype(np.int32)
    in_maps = []
    for c in range(NCORES):
        basec_c = ((c * P + np.arange(P)) * BPP).astype(np.float32)[:, None]
        in_maps.append({
            "psrc": psrc_a[c], "pdst": pdst_a[c], "sdst": sdst_a[c],
            "wb": wb, "mmat": mmat, "basec": basec_c,
            "iota16": iota_a, "smask": smask_a,
        })
    return in_maps, order1, counts1


def kernel(edge_index, num_nodes, W, b):
    global _CACHE
    if "nc" not in _CACHE:
        _CACHE["nc"] = _build()
    nc = _CACHE["nc"]

    in_maps, order1, counts1 = _host_prep(edge_index, W, b)
    res = run_bass_kernel_spmd(nc, in_maps, list(range(NCORES)))

    E = np.asarray(edge_index[0]).shape[0]
    out_full = np.empty((E, EMB), np.float32)
    rows = []
    for c in range(NCORES):
        o = res.results[c]["out"]          # [P, NX, EMB, XCH] f16
        o = np.ascontiguousarray(o.transpose(0, 1, 3, 2)).reshape(P, T, EMB)
        for p in range(P):
            n = counts1[c, p]
            if n:
                rows.append(o[p, :n, :])
    out_full[order1] = np.concatenate(rows, axis=0).astype(np.float32)
    return out_full


# revision 12
# speedup vs baseline: 5.8879x; 2.4141x over previous
"""DegreeAwareEdgeEncoder Trainium2 kernel (8 NeuronCores, Bass/Tile).

Sharding strategy (host side, inside kernel()):
  Edge-parallel: core c owns the edges whose src falls in its 12800-node
  range.  Two views of each core's OWN edges are delivered:
    - psrc: bucketed by src into 128 partition slabs (100 nodes each),
      sorted by src (vertex-range partitioning).
    - sdst2: bucketed by dst into 128 partition rows (784 nodes each),
      sorted by dst.
  All arithmetic happens on the device:
    - out-degree per edge (du): log2 shifted max/min window over the sorted
      src runs; du = end - start + 1.
    - in-degree: local degree partials per node via run-boundary scatters
      (GPSIMD local_scatter) on the dst-sorted view, AllReduce of the
      [num_nodes] partial vectors across the 8 cores, then per-edge
      expansion by placing the reduced degree at each run start and
      propagating it along the run with a shifted-max window (no per-edge
      gather).
    - route: the per-edge dv values are permuted from the dst-sorted view
      back to the src-slab output slots entirely on-device: within-row
      local_scatter -> 52x 128x128 DMA transposes -> within-row
      local_scatter (host precomputes the routing indices; pure layout).
    - output rows: du*A' + dv*B' + b with A'=W0+W2, B'=W1+W2 (PE computes
      the coefficient rows), expanded in fp16 [P, EMB, chunk] tiles.
  The host only buckets/sorts (data layout), pads with sentinel edges,
  precomputes routing index maps, and inverts the layout permutation on
  the returned rows.
"""

import numpy as np

import concourse.bass as bass
import concourse.mybir as mybir
import concourse.tile as tile
from concourse.tile_rust import add_dep_helper
from concourse import bacc
from concourse.library_config import local_scatter as LS_LIB
from concourse.bass_utils import run_bass_kernel_spmd

# ---- constants ----
N_NODES = 100_000
N_EDGES = 3_200_000
EMB = 32
NCORES = 8
P = 128
BPP = 100                  # src nodes per partition slab
BP2 = 784                  # dst nodes per partition row (dst2 view)
NV2 = P * BP2              # 100352 virtual dst nodes
RC = P * BPP               # 12800 src nodes per core
T = 3584                   # slab capacity (cols per partition)
NB = 52                    # route blocks (capacity per (row, target-row))
G3 = NB * P                # 6656 route grid cols
WS = 2046                  # local_scatter output window (must be < 2048, even)
NW_A = 4                   # windows covering G3 (4*2046 = 8184)
NW_C = 2                   # windows covering T (2*2046 = 4092)
DV2W = NW_C * WS           # 4092
XCH = 64                   # expansion chunk cols
NX = T // XCH              # 56 expansion chunks
BIG = 65536.0

f32 = mybir.dt.float32
f16 = mybir.dt.float16
i32 = mybir.dt.int32
i16 = mybir.dt.int16
AO = mybir.AluOpType
AX = mybir.AxisListType

_CACHE = {}


def _build():
    nc = bacc.Bacc("TRN2", target_bir_lowering=False, debug=False,
                   num_devices=NCORES)

    psrc = nc.dram_tensor("psrc", [P, T], i32, kind="ExternalInput")
    sdst2 = nc.dram_tensor("sdst2", [P, T], i32, kind="ExternalInput")
    wb_in = nc.dram_tensor("wb", [4, EMB], f32, kind="ExternalInput")
    mmat = nc.dram_tensor("mmat", [4, 4], f32, kind="ExternalInput")
    basec = nc.dram_tensor("basec", [P, 1], f32, kind="ExternalInput")
    basec2 = nc.dram_tensor("basec2", [P, 1], f32, kind="ExternalInput")
    iota16 = nc.dram_tensor("iota16", [P, T], i16, kind="ExternalInput")
    s1idx = nc.dram_tensor("s1idx", [NW_A, P, T], i16, kind="ExternalInput")
    s3idx = nc.dram_tensor("s3idx", [NW_C, P, G3], i16, kind="ExternalInput")
    out = nc.dram_tensor("out", [P, NX, EMB, XCH], f16, kind="ExternalOutput")

    partial_d = nc.dram_tensor("partial_d", [NV2], f32)
    allred_d = nc.dram_tensor("allred_d", [NV2], f32, addr_space="Shared")
    abb_d = nc.dram_tensor("abb_d", [4, EMB], f16)

    with tile.TileContext(nc) as tc, nc.allow_low_precision(
            reason="degrees are small integers, exact in fp16; fp16 products "
                   "are within the 2e-2 relative-error gate"):
        with (
            tc.tile_pool(name="persist", bufs=1) as pp,
            tc.tile_pool(name="psum", bufs=1, space="PSUM") as psum,
        ):
            # ---- persistent tiles / constants ----
            wb_t = pp.tile([4, EMB], f32)
            mm_t = pp.tile([4, 4], f32)
            basec_t = pp.tile([P, 1], f32)
            basec2_t = pp.tile([P, 1], f32)
            nc.sync.dma_start(out=wb_t[:], in_=wb_in[:])
            nc.sync.dma_start(out=mm_t[:], in_=mmat[:])
            nc.sync.dma_start(out=basec_t[:], in_=basec[:])
            nc.sync.dma_start(out=basec2_t[:], in_=basec2[:])
            iota16_t = pp.tile([P, T], i16)
            nc.sync.dma_start(out=iota16_t[:], in_=iota16[:])

            # coefficient rows: [A'; B'; b; 0] = mmat^T @ [W; b]
            abb_ps = psum.tile([4, EMB], f32)
            nc.tensor.matmul(out=abb_ps[:], lhsT=mm_t[:], rhs=wb_t[:],
                             start=True, stop=True)
            abb_h = pp.tile([4, EMB], f16)
            nc.vector.tensor_copy(out=abb_h[:], in_=abb_ps[:])
            nc.sync.dma_start(out=abb_d[:], in_=abb_h[:])
            arow = pp.tile([P, EMB], f16)
            brow = pp.tile([P, EMB], f16)
            crow = pp.tile([P, EMB], f16)
            nc.sync.dma_start(out=arow[:], in_=abb_d[0:1, :].to_broadcast([P, EMB]))
            nc.sync.dma_start(out=brow[:], in_=abb_d[1:2, :].to_broadcast([P, EMB]))
            nc.sync.dma_start(out=crow[:], in_=abb_d[2:3, :].to_broadcast([P, EMB]))
            arep = pp.tile([P, EMB, XCH], f16)
            brep = pp.tile([P, EMB, XCH], f16)
            crep = pp.tile([P, EMB, XCH], f16)
            nc.vector.tensor_copy(
                out=arep[:], in_=arow[:][:, :, None].to_broadcast([P, EMB, XCH]))
            nc.vector.tensor_copy(
                out=brep[:], in_=brow[:][:, :, None].to_broadcast([P, EMB, XCH]))
            nc.vector.tensor_copy(
                out=crep[:], in_=crow[:][:, :, None].to_broadcast([P, EMB, XCH]))

            du_h = pp.tile([P, T], f16)
            dvd = pp.tile([P, T], f16)
            dv2 = pp.tile([P, DV2W], f16)
            brk2 = pp.tile([P, T], f32)
            FstCol = pp.tile([P, BP2], i16)

            ls_lib = nc.gpsimd.load_library(LS_LIB)

            with tc.tile_pool(name="pe", bufs=1) as pe:
                # ==== dst2 phase: local in-degree partials ====
                sdst2_t = pe.tile([P, T], i32, tag="A")
                nc.sync.dma_start(out=sdst2_t[:], in_=sdst2[:])
                v2 = pe.tile([P, T], f32, tag="B")
                nc.vector.tensor_copy(out=v2[:], in_=sdst2_t[:])
                nc.vector.scalar_tensor_tensor(
                    out=v2[:], in0=v2[:], scalar=basec2_t[:, 0:1],
                    in1=v2[:], op0=AO.subtract, op1=AO.bypass)
                nc.vector.memset(brk2[:, 0:1], 1.0)
                nc.vector.tensor_tensor(out=brk2[:, 1:], in0=v2[:, 1:],
                                        in1=v2[:, :T - 1], op=AO.not_equal)
                # data for both boundary scatters: t + 1
                Ldata = pe.tile([P, T], i16, tag="Ld")
                nc.vector.tensor_scalar(out=Ldata[:], in0=iota16_t[:],
                                        scalar1=1, scalar2=None, op0=AO.add)
                # first-of-run indices
                tmpF = pe.tile([P, T], f32, tag="D")
                nc.vector.tensor_tensor(out=tmpF[:], in0=v2[:], in1=brk2[:],
                                        op=AO.mult)
                nc.vector.tensor_tensor(out=tmpF[:], in0=tmpF[:], in1=brk2[:],
                                        op=AO.add)
                nc.vector.tensor_scalar(out=tmpF[:], in0=tmpF[:], scalar1=-1.0,
                                        scalar2=None, op0=AO.add)
                idxF = pe.tile([P, T], i16, tag="E")
                nc.vector.tensor_copy(out=idxF[:], in_=tmpF[:])
                Fst1 = pe.tile([P, BP2], i16)
                s1 = nc.gpsimd.local_scatter(Fst1[:], Ldata[:], idxF[:],
                                             P, BP2, T)
                add_dep_helper(s1.ins, ls_lib.ins, sync=True,
                               reason="local_scatter needs library loaded")
                # last-of-run indices
                brkL = pe.tile([P, T], f32, tag="F")
                nc.vector.memset(brkL[:, T - 1:T], 1.0)
                nc.vector.tensor_tensor(out=brkL[:, :T - 1], in0=v2[:, :T - 1],
                                        in1=v2[:, 1:], op=AO.not_equal)
                tmpL = pe.tile([P, T], f32, tag="D")
                nc.vector.tensor_tensor(out=tmpL[:], in0=v2[:], in1=brkL[:],
                                        op=AO.mult)
                nc.vector.tensor_tensor(out=tmpL[:], in0=tmpL[:], in1=brkL[:],
                                        op=AO.add)
                nc.vector.tensor_scalar(out=tmpL[:], in0=tmpL[:], scalar1=-1.0,
                                        scalar2=None, op0=AO.add)
                idxL = pe.tile([P, T], i16, tag="G")
                nc.vector.tensor_copy(out=idxL[:], in_=tmpL[:])
                Lst1 = pe.tile([P, BP2], i16)
                s2 = nc.gpsimd.local_scatter(Lst1[:], Ldata[:], idxL[:],
                                             P, BP2, T)
                add_dep_helper(s2.ins, ls_lib.ins, sync=True,
                               reason="local_scatter needs library loaded")
                # partial degree = Lst1 - Fst1 + (Fst1 > 0); FstCol = Fst1 - 1
                pL = pe.tile([P, BP2], f32)
                pF = pe.tile([P, BP2], f32)
                nc.vector.tensor_copy(out=pL[:], in_=Lst1[:])
                nc.vector.tensor_copy(out=pF[:], in_=Fst1[:])
                nc.vector.tensor_tensor(out=pL[:], in0=pL[:], in1=pF[:],
                                        op=AO.subtract)
                pM = pe.tile([P, BP2], f32)
                nc.vector.tensor_scalar(out=pM[:], in0=pF[:], scalar1=0.0,
                                        scalar2=None, op0=AO.is_gt)
                nc.vector.tensor_tensor(out=pL[:], in0=pL[:], in1=pM[:],
                                        op=AO.add)
                nc.vector.tensor_scalar(out=pF[:], in0=pF[:], scalar1=-1.0,
                                        scalar2=None, op0=AO.add)
                nc.vector.tensor_copy(out=FstCol[:], in_=pF[:])
                nc.sync.dma_start(
                    out=partial_d[:].rearrange("(p c) -> p c", p=P), in_=pL[:])
                nc.gpsimd.collective_compute(
                    "AllReduce", AO.add,
                    replica_groups=[list(range(NCORES))],
                    ins=[partial_d[:]], outs=[allred_d[:]])

                # ==== src phase: du per edge via run windows ====
                psrc_t = pe.tile([P, T], i32, tag="A")
                nc.sync.dma_start(out=psrc_t[:], in_=psrc[:])
                vsf = pe.tile([P, T], f32, tag="B")
                nc.vector.tensor_copy(out=vsf[:], in_=psrc_t[:])
                nc.vector.scalar_tensor_tensor(
                    out=vsf[:], in0=vsf[:], scalar=basec_t[:, 0:1],
                    in1=vsf[:], op0=AO.subtract, op1=AO.bypass)
                iotaF = pe.tile([P, T], f32, tag="H")
                nc.vector.tensor_copy(out=iotaF[:], in_=iota16_t[:])
                brkS = pe.tile([P, T], f32, tag="Ld")
                nc.vector.memset(brkS[:, 0:1], 1.0)
                nc.vector.tensor_tensor(out=brkS[:, 1:], in0=vsf[:, 1:],
                                        in1=vsf[:, :T - 1], op=AO.not_equal)
                sa = pe.tile([P, T], f32, tag="D")
                sb = pe.tile([P, T], f32, tag="E")
                nc.vector.tensor_scalar(out=sa[:], in0=brkS[:], scalar1=BIG,
                                        scalar2=-BIG, op0=AO.mult, op1=AO.add)
                nc.vector.tensor_tensor(out=sb[:], in0=iotaF[:], in1=brkS[:],
                                        op=AO.mult)
                nc.vector.tensor_tensor(out=sa[:], in0=sa[:], in1=sb[:],
                                        op=AO.add)
                cur, nxt = sa, sb
                for s in (1, 2, 4, 8, 16, 32, 64):
                    nc.vector.tensor_tensor(out=nxt[:, s:], in0=cur[:, s:],
                                            in1=cur[:, :T - s], op=AO.max)
                    nc.vector.tensor_copy(out=nxt[:, :s], in_=cur[:, :s])
                    cur, nxt = nxt, cur
                S_fin = cur
                brkE = pe.tile([P, T], f32, tag="Ld")
                nc.vector.memset(brkE[:, T - 1:T], 1.0)
                nc.vector.tensor_tensor(out=brkE[:, :T - 1],
                                        in0=vsf[:, :T - 1],
                                        in1=vsf[:, 1:], op=AO.not_equal)
                ea = pe.tile([P, T], f32, tag="F")
                eb = pe.tile([P, T], f32, tag="G")
                nc.vector.tensor_scalar(out=ea[:], in0=brkE[:], scalar1=-BIG,
                                        scalar2=BIG, op0=AO.mult, op1=AO.add)
                nc.vector.tensor_tensor(out=eb[:], in0=iotaF[:], in1=brkE[:],
                                        op=AO.mult)
                nc.vector.tensor_tensor(out=ea[:], in0=ea[:], in1=eb[:],
                                        op=AO.add)
                cur, nxt = ea, eb
                for s in (1, 2, 4, 8, 16, 32, 64):
                    nc.vector.tensor_tensor(out=nxt[:, :T - s],
                                            in0=cur[:, :T - s],
                                            in1=cur[:, s:], op=AO.min)
                    nc.vector.tensor_copy(out=nxt[:, T - s:],
                                          in_=cur[:, T - s:])
                    cur, nxt = nxt, cur
                E_fin = cur
                duf = pe.tile([P, T], f32, tag="B")
                nc.vector.tensor_tensor(out=duf[:], in0=E_fin[:],
                                        in1=S_fin[:], op=AO.subtract)
                nc.vector.tensor_scalar(out=du_h[:], in0=duf[:], scalar1=1.0,
                                        scalar2=None, op0=AO.add)

            with tc.tile_pool(name="pl", bufs=1) as pl:
                # ==== placement: full degree at run starts + propagate ====
                FULLDEG = pl.tile([P, BP2], f32)
                nc.sync.dma_start(
                    out=FULLDEG[:],
                    in_=allred_d[:].rearrange("(p c) -> p c", p=P))
                FULLDEG16 = pl.tile([P, BP2], i16)
                nc.vector.tensor_copy(out=FULLDEG16[:], in_=FULLDEG[:])
                placed = pl.tile([P, DV2W], i16)
                for w in range(NW_C):
                    fw = pl.tile([P, BP2], f32, tag="fw")
                    nc.vector.tensor_scalar(out=fw[:], in0=FstCol[:],
                                            scalar1=float(-w * WS),
                                            scalar2=None, op0=AO.add)
                    fm = pl.tile([P, BP2], f32, tag="fm")
                    nc.vector.tensor_scalar(out=fm[:], in0=fw[:],
                                            scalar1=float(WS), scalar2=None,
                                            op0=AO.is_lt)
                    fg = pl.tile([P, BP2], f32, tag="fg")
                    nc.vector.tensor_scalar(out=fg[:], in0=fw[:], scalar1=0.0,
                                            scalar2=None, op0=AO.is_ge)
                    nc.vector.tensor_tensor(out=fm[:], in0=fm[:], in1=fg[:],
                                            op=AO.mult)
                    nc.vector.tensor_tensor(out=fw[:], in0=fw[:], in1=fm[:],
                                            op=AO.mult)
                    nc.vector.tensor_tensor(out=fw[:], in0=fw[:], in1=fm[:],
                                            op=AO.add)
                    nc.vector.tensor_scalar(out=fw[:], in0=fw[:],
                                            scalar1=-1.0, scalar2=None,
                                            op0=AO.add)
                    fwi = pl.tile([P, BP2], i16, tag="fwi")
                    nc.vector.tensor_copy(out=fwi[:], in_=fw[:])
                    sc = nc.gpsimd.local_scatter(
                        placed[:, w * WS:(w + 1) * WS], FULLDEG16[:], fwi[:],
                        P, WS, BP2)
                    add_dep_helper(sc.ins, ls_lib.ins, sync=True,
                                   reason="local_scatter needs library loaded")
                iotaG = pl.tile([P, T], f32, tag="D")
                nc.vector.tensor_copy(out=iotaG[:], in_=iota16_t[:])
                placed_f = pl.tile([P, T], f32, tag="E")
                nc.vector.tensor_copy(out=placed_f[:], in_=placed[:, :T])
                # m0 = brk2*(iota*256 + placed + BIG) - BIG
                ma = pl.tile([P, T], f32, tag="F")
                mb = pl.tile([P, T], f32, tag="G")
                nc.vector.tensor_scalar(out=ma[:], in0=iotaG[:], scalar1=256.0,
                                        scalar2=BIG, op0=AO.mult, op1=AO.add)
                nc.vector.tensor_tensor(out=ma[:], in0=ma[:], in1=placed_f[:],
                                        op=AO.add)
                nc.vector.tensor_tensor(out=ma[:], in0=ma[:], in1=brk2[:],
                                        op=AO.mult)
                nc.vector.tensor_scalar(out=ma[:], in0=ma[:], scalar1=-BIG,
                                        scalar2=None, op0=AO.add)
                cur, nxt = ma, mb
                for s in (1, 2, 4, 8, 16):
                    nc.vector.tensor_tensor(out=nxt[:, s:], in0=cur[:, s:],
                                            in1=cur[:, :T - s], op=AO.max)
                    nc.vector.tensor_copy(out=nxt[:, :s], in_=cur[:, :s])
                    cur, nxt = nxt, cur
                M_fin = cur
                # dv = m - 256*floor-ish(m/256)
                hi = pl.tile([P, T], f32, tag="D")
                nc.vector.tensor_scalar(out=hi[:], in0=M_fin[:],
                                        scalar1=1.0 / 256.0, scalar2=-0.499,
                                        op0=AO.mult, op1=AO.add)
                hii = pl.tile([P, T], i16, tag="hii")
                nc.vector.tensor_copy(out=hii[:], in_=hi[:])
                nc.vector.tensor_copy(out=hi[:], in_=hii[:])
                dvf = pl.tile([P, T], f32, tag="E")
                nc.vector.scalar_tensor_tensor(
                    out=dvf[:], in0=hi[:], scalar=-256.0, in1=M_fin[:],
                    op0=AO.mult, op1=AO.add)
                nc.vector.tensor_copy(out=dvd[:], in_=dvf[:])

                # ==== route: dst2 layout -> src slab layout ====
                grid3 = pl.tile([P, G3], f16, tag="F")
                for w in range(NW_A):
                    lo = w * WS
                    hi_c = min((w + 1) * WS, G3)
                    s1w = pl.tile([P, T], i16, tag=f"s1w{w % 2}")
                    nc.sync.dma_start(out=s1w[:], in_=s1idx[w])
                    sc = nc.gpsimd.local_scatter(
                        grid3[:, lo:hi_c], dvd[:], s1w[:], P, hi_c - lo, T)
                    add_dep_helper(sc.ins, ls_lib.ins, sync=True,
                                   reason="local_scatter needs library loaded")
                grid4 = pl.tile([P, G3], f16, tag="G")
                for b in range(NB):
                    nc.sync.dma_start_transpose(
                        out=grid4[:, b * P:(b + 1) * P],
                        in_=grid3[:, b * P:(b + 1) * P])
                for w in range(NW_C):
                    s3w = pl.tile([P, G3], i16, tag=f"s3w{w % 2}")
                    nc.sync.dma_start(out=s3w[:], in_=s3idx[w])
                    sc = nc.gpsimd.local_scatter(
                        dv2[:, w * WS:(w + 1) * WS], grid4[:], s3w[:],
                        P, WS, G3)
                    add_dep_helper(sc.ins, ls_lib.ins, sync=True,
                                   reason="local_scatter needs library loaded")

                # ==== expansion: out = du*A' + dv*B' + b ====
                for x in range(NX):
                    sl = slice(x * XCH, (x + 1) * XCH)
                    x1 = pl.tile([P, EMB, XCH], f16, tag=f"ex{x % 2}")
                    x2 = pl.tile([P, EMB, XCH], f16, tag="ey")
                    nc.vector.tensor_tensor(
                        out=x1[:],
                        in0=du_h[:, sl][:, None, :].to_broadcast(
                            [P, EMB, XCH]),
                        in1=arep[:], op=AO.mult)
                    nc.vector.tensor_tensor(
                        out=x2[:],
                        in0=dv2[:, sl][:, None, :].to_broadcast(
                            [P, EMB, XCH]),
                        in1=brep[:], op=AO.mult)
                    nc.vector.tensor_tensor(out=x1[:], in0=x1[:], in1=x2[:],
                                            op=AO.add)
                    nc.vector.tensor_tensor(out=x1[:], in0=x1[:],
                                            in1=crep[:], op=AO.add)
                    nc.scalar.dma_start(out=out[:, x, :, :], in_=x1[:])

    nc.compile()
    return nc


def _host_prep(edge_index, W, b):
    src = np.asarray(edge_index[0], dtype=np.int64).astype(np.int32)
    dst = np.asarray(edge_index[1], dtype=np.int64).astype(np.int32)
    E = src.shape[0]

    # ---- src-slab layout (output layout) ----
    order1 = np.argsort(src, kind="stable")
    k_s = src[order1]
    part = (k_s // BPP).astype(np.int64)           # 0..1023 global partition
    counts1 = np.bincount(part, minlength=NCORES * P)
    if counts1.max() > T:
        raise RuntimeError(f"src slab overflow: {counts1.max()} > {T}")
    starts = np.zeros(NCORES * P + 1, np.int64)
    np.cumsum(counts1, out=starts[1:])
    pos1 = np.arange(E, dtype=np.int64) - starts[part]   # t1 (sorted order)
    psrc_a = np.full((NCORES * P, T), 0, np.int32)
    gp = np.arange(NCORES * P, dtype=np.int32)
    psrc_a[:] = (gp * BPP + BPP)[:, None]          # pad sentinel v=BPP
    psrc_a[part, pos1] = k_s
    psrc_a = psrc_a.reshape(NCORES, P, T)

    # per-edge (original order) core / p1 / t1
    coreid = np.empty(E, np.int64)
    p1 = np.empty(E, np.int64)
    t1 = np.empty(E, np.int64)
    coreid[order1] = part // P
    p1[order1] = part % P
    t1[order1] = pos1

    # ---- dst2 layout (per-core own edges, dst-range rows) ----
    p0 = (dst // BP2).astype(np.int64)             # 0..127
    key2 = (coreid * P + p0) * (N_NODES + 1) + dst
    order2 = np.argsort(key2, kind="stable")
    row2 = (coreid * P + p0)[order2]               # 0..1023 (core-major)
    counts2 = np.bincount(row2, minlength=NCORES * P)
    if counts2.max() > T:
        raise RuntimeError(f"dst2 row overflow: {counts2.max()} > {T}")
    starts2 = np.zeros(NCORES * P + 1, np.int64)
    np.cumsum(counts2, out=starts2[1:])
    pos2 = np.arange(E, dtype=np.int64) - starts2[row2]  # t0 (sorted order)
    sdst2_a = np.empty((NCORES * P, T), np.int32)
    sdst2_a[:] = (np.arange(NCORES * P, dtype=np.int32) % P * BP2 - 1)[:, None]
    sdst2_a[row2, pos2] = dst[order2]
    sdst2_a = sdst2_a.reshape(NCORES, P, T)

    # sanity: partial (per-core) degree fits the 5-step propagate window
    pr = np.bincount(coreid * N_NODES + dst.astype(np.int64),
                     minlength=NCORES * N_NODES)
    if pr.max() > 31:
        raise RuntimeError(f"partial degree {pr.max()} > 31")

    # ---- routing indices ----
    # stage A: element (p0, t0) -> grid3 col b*128 + p1_e ; b = rank within
    # (row2, p1) group.  stage C: grid4[p1, b*128 + p0] -> t1.
    e2 = order2                                    # edge ids in dst2 order
    p0l = row2 % P                                 # dst2 row within core
    core2 = row2 // P
    rkey = row2 * P + p1[e2]                       # (row2, target-row) group
    rord = np.argsort(rkey, kind="stable")
    rstarts_full = np.zeros(NCORES * P * P + 1, np.int64)
    np.cumsum(np.bincount(rkey, minlength=NCORES * P * P),
              out=rstarts_full[1:])
    bb = np.empty(E, np.int64)
    bb[rord] = np.arange(E) - rstarts_full[rkey[rord]]
    if bb.max() >= NB:
        raise RuntimeError(f"route block overflow: {bb.max()} >= {NB}")
    col3 = bb * P + p1[e2]                         # grid3 col per dst2-elem
    s1_a = np.full((NCORES, NW_A, P, T), -1, np.int16)
    w = col3 // WS
    s1_a[core2, w, p0l, pos2] = (col3 - w * WS).astype(np.int16)
    # stage C: grid4 position (p1, bb*128 + p0)
    s3_a = np.full((NCORES, NW_C, P, G3), -1, np.int16)
    wc = t1[e2] // WS
    s3_a[core2, wc, p1[e2], bb * P + p0l] = (t1[e2] - wc * WS).astype(np.int16)

    wb = np.concatenate([np.asarray(W, np.float32),
                         np.asarray(b, np.float32)[None, :]], axis=0)
    mmat = np.array([[1, 0, 0, 0],
                     [0, 1, 0, 0],
                     [1, 1, 0, 0],
                     [0, 0, 1, 0]], np.float32)
    iota_a = np.tile(np.arange(T, dtype=np.int16), (P, 1))
    in_maps = []
    for c in range(NCORES):
        basec_c = ((c * P + np.arange(P)) * BPP).astype(np.float32)[:, None]
        basec2_c = (np.arange(P) * BP2).astype(np.float32)[:, None]
        in_maps.append({
            "psrc": psrc_a[c], "sdst2": sdst2_a[c],
            "wb": wb, "mmat": mmat, "basec": basec_c, "basec2": basec2_c,
            "iota16": iota_a, "s1idx": s1_a[c], "s3idx": s3_a[c],
        })
    return in_maps, order1, counts1.reshape(NCORES, P)


def kernel(edge_index, num_nodes, W, b):
    global _CACHE
    if "nc" not in _CACHE:
        _CACHE["nc"] = _build()
    nc = _CACHE["nc"]

    in_maps, order1, counts1 = _host_prep(edge_index, W, b)
    res = run_bass_kernel_spmd(nc, in_maps, list(range(NCORES)))

    E = np.asarray(edge_index[0]).shape[0]
    out_full = np.empty((E, EMB), np.float32)
    rows = []
    for c in range(NCORES):
        o = res.results[c]["out"]          # [P, NX, EMB, XCH] f16
        o = np.ascontiguousarray(o.transpose(0, 1, 3, 2)).reshape(P, T, EMB)
        for p in range(P):
            n = counts1[c, p]
            if n:
                rows.append(o[p, :n, :])
    out_full[order1] = np.concatenate(rows, axis=0).astype(np.float32)
    return out_full
